# revision 7
# baseline (speedup 1.0000x reference)
"""BitLinear (ternary-weight linear) Trainium2 kernel — fp8 DoubleRow version.

Math (matching the reference):
    s      = max(act_scale, 1e-5)
    z      = clip(round(x / s), -127, 127)           # int8-valued
    out    = (alpha * s) * (z @ sign(W).T) + bias

Key idea: TRN2's fp8 DoubleRow matmul contracts 2 k-tiles (256 deep) per
instruction at 0.5 cycles per output row -> 4x bf16 MAC throughput. z in
[-127,127] is not exact in fp8e4 (4 sig bits), so split exactly:
    h   = round(z / 16)        (any rounding mode works)
    h16 = 16 * h               in {-128..128, step 16}  -> exact in fp8e4
    l   = z - h16              in [-15, 15] integers    -> exact in fp8e4
    z @ W = h16 @ W + l @ W    (both passes accumulate into the same PSUM
                                bank; f32 accumulator keeps integer
                                arithmetic exact)
Two DoubleRow passes = 2x bf16 matmul throughput overall: ~437us of PE
time per core vs the bf16 baseline's ~874us.

Device strategy (8 cores, data-parallel over the 16384 tokens, 2048 each):
  - Quantize x token-major (ACT round+scale, DVE clamp) -> z int16, bounce
    through DRAM, xbar-DMA-transpose (2-byte granularity) back k-major.
  - Convert zT -> h16/l fp8 tiles on ACT/DVE/Pool (layout- and
    rounding-mode-robust: any h with |z-16h|<=15 recombines exactly).
  - Weights replicated, host-packed to fp8e4 [128, NT, KT, 512]; streamed
    once (16MB) as one n-chunk at a time, double buffered.
  - PSUM drain fused on DVE: out = psum * (alpha*s) + bias -> bf16 store
    (bf16 output rounding ~1.3e-3 rel, well within 2e-2; host upcasts).
"""

import sys

sys.path.insert(0, "/opt/trn_rl_repo")

import numpy as np
import ml_dtypes

# ---- problem constants (hardcoded per harness contract) ----
B, S, IN, OUT = 4, 4096, 4096, 4096
TOKENS = B * S              # 16384
N_CORES = 8
T = TOKENS // N_CORES       # 2048 tokens per core
KT = IN // 128              # 32 k-tiles (contraction)
KP = KT // 2                # 16 DoubleRow k-pairs
N_CHUNK = 512               # output columns per PSUM tile
NT = OUT // N_CHUNK         # 8 n-chunks
Q = 512                     # token-quarter (transpose/convert granularity)
NQ = T // Q                 # 4 quarters
MT = Q // 128               # 4 m-tiles per quarter
XCH = 1024                  # free-dim chunk for quantization staging


def _build_program(inv_s: float, alpha_s: float, reps: int = 1,
                   bonly: bool = False):
    import concourse.mybir as mybir
    import concourse.tile as tile
    from concourse import bacc

    nc = bacc.Bacc("TRN2", target_bir_lowering=False, debug=False,
                   num_devices=N_CORES)

    x_d = nc.dram_tensor("x", [T, IN], mybir.dt.float32, kind="ExternalInput")
    # wt[p, n, k, c] = sign(W)[n*512 + c, k*128 + p]
    wt_d = nc.dram_tensor("wt", [128, NT, KT, N_CHUNK], mybir.dt.float8e4,
                          kind="ExternalInput")
    bias_d = nc.dram_tensor("bias", [128, OUT], mybir.dt.bfloat16,
                            kind="ExternalInput")
    out_d = nc.dram_tensor("out", [T, OUT], mybir.dt.bfloat16,
                           kind="ExternalOutput")
    z_d = nc.dram_tensor("z_scratch", [T, IN], mybir.dt.int16)

    AF = mybir.ActivationFunctionType
    ALU = mybir.AluOpType
    DR = mybir.MatmulPerfMode.DoubleRow

    with tile.TileContext(nc) as tc:
        with (
            tc.tile_pool(name="xstage", bufs=2) as xstage,
            tc.tile_pool(name="zstage", bufs=2) as zstage,
            tc.tile_pool(name="ztp", bufs=4) as ztp,
            tc.tile_pool(name="hp", bufs=4) as hp,
            tc.tile_pool(name="hl", bufs=NQ) as hl_pool,
            tc.tile_pool(name="wtp", bufs=2) as wt_pool,
            tc.tile_pool(name="outsb", bufs=4) as out_pool,
            tc.tile_pool(name="biasp", bufs=1) as bias_pool,
            tc.tile_pool(name="psum", bufs=8, space="PSUM") as psum_pool,
        ):
            bias_t = bias_pool.tile([128, OUT], mybir.dt.bfloat16, tag="bias")
            nc.scalar.dma_start(bias_t[:], bias_d.ap())

            # DMA ring assignment: SP (nc.sync) carries the xbar transposes;
            # Pool/SWDGE (nc.gpsimd) carries x loads; ACT (nc.scalar)
            # carries z stores, weight loads, bias load and output stores.
            def emit_quant(m):
                """Quantize one 128-token row block: x -> round/clip ->
                int16, bounce to DRAM."""
                r0 = m * 128
                for c in range(IN // XCH):
                    i0 = c * XCH
                    xt = xstage.tile([128, XCH], mybir.dt.float32, tag="xf32")
                    nc.gpsimd.dma_start(xt[:],
                                        x_d.ap()[r0:r0 + 128, i0:i0 + XCH])
                    z0 = zstage.tile([128, XCH], mybir.dt.int16, tag="z0")
                    nc.scalar.activation(z0[:], xt[:], AF.Copy,
                                         bias=0.0, scale=float(inv_s))
                    z1 = zstage.tile([128, XCH], mybir.dt.int16, tag="z1")
                    nc.vector.tensor_scalar(z1[:], z0[:], 127.0, -127.0,
                                            ALU.min, ALU.max)
                    nc.scalar.dma_start(z_d.ap()[r0:r0 + 128, i0:i0 + XCH],
                                        z1[:])

            def emit_convert(q):
                """Transpose quarter q k-major and split into exact fp8
                h16/l tiles. Returns (h16, l) tiles [128, KT, Q]."""
                t0 = q * Q
                h16_t = hl_pool.tile([128, KT, Q], mybir.dt.float8e4,
                                     tag="h16")
                l_t = hl_pool.tile([128, KT, Q], mybir.dt.float8e4, tag="l")
                for k in range(KT):
                    zT = ztp.tile([128, Q], mybir.dt.int16, tag="zT")
                    nc.sync.dma_start_transpose(
                        zT[:], z_d.ap()[t0:t0 + Q, k * 128:(k + 1) * 128])
                    h = hp.tile([128, Q], mybir.dt.int16, tag="h")
                    nc.scalar.activation(h[:], zT[:], AF.Copy,
                                         bias=0.0, scale=0.0625)
                    nc.vector.tensor_scalar(h16_t[:, k, :], h[:], 16.0, None,
                                            ALU.mult)
                    nc.gpsimd.tensor_tensor(l_t[:, k, :], zT[:],
                                            h16_t[:, k, :], ALU.subtract)
                return h16_t, l_t

            hls = None
            for _rep in range(reps):
                if hls is None or not bonly:
                    hls = []
                    for q in range(NQ):
                        for mm in range(MT):
                            emit_quant(q * MT + mm)
                        hls.append(emit_convert(q))

                for n in range(NT):
                    wt = wt_pool.tile([128, KT, N_CHUNK], mybir.dt.float8e4,
                                      tag="wt")
                    nc.scalar.dma_start(wt[:], wt_d.ap()[:, n, :, :])
                    for q in range(NQ):
                        h16_t, l_t = hls[q]
                        for mm in range(MT):
                            ms = slice(mm * 128, (mm + 1) * 128)
                            psum = psum_pool.tile([128, N_CHUNK],
                                                  mybir.dt.float32)
                            for kk in range(KP):
                                ks = slice(2 * kk, 2 * kk + 2)
                                nc.tensor.matmul(
                                    psum[:], h16_t[:, ks, ms], wt[:, ks, :],
                                    start=(kk == 0), stop=False,
                                    perf_mode=DR)
                            for kk in range(KP):
                                ks = slice(2 * kk, 2 * kk + 2)
                                nc.tensor.matmul(
                                    psum[:], l_t[:, ks, ms], wt[:, ks, :],
                                    start=False, stop=(kk == KP - 1),
                                    perf_mode=DR)
                            osb = out_pool.tile([128, N_CHUNK],
                                                mybir.dt.bfloat16, tag="osb")
                            nc.vector.scalar_tensor_tensor(
                                osb[:], psum[:], float(alpha_s),
                                bias_t[:, n * N_CHUNK:(n + 1) * N_CHUNK],
                                ALU.mult, ALU.add)
                            t0 = q * Q + mm * 128
                            nc.scalar.dma_start(
                                out_d.ap()[t0:t0 + 128,
                                           n * N_CHUNK:(n + 1) * N_CHUNK],
                                osb[:])

    nc.compile()
    return nc


def prep_scalars(alpha, act_scale):
    s = max(float(np.asarray(act_scale)), 1e-5)
    inv_s = 1.0 / np.float32(s)
    alpha_s = float(np.float32(np.asarray(alpha, dtype=np.float32)) *
                    np.float32(s))
    return float(inv_s), alpha_s


def prep_weights(packed_w, bias):
    """Host-side weight/bias packing (replicated across cores)."""
    w_sign = np.asarray(packed_w, dtype=np.float32) - 1.0     # [OUT, IN]
    # wt[p, n, k, c] = w_sign[n*512 + c, k*128 + p]
    wt = w_sign.reshape(NT, N_CHUNK, KT, 128).transpose(3, 0, 2, 1)
    whost = np.ascontiguousarray(wt.astype(ml_dtypes.float8_e4m3))
    bias_rep = np.ascontiguousarray(
        np.broadcast_to(
            np.asarray(bias, dtype=np.float32).astype(ml_dtypes.bfloat16)
            [None, :], (128, OUT)))
    return whost, bias_rep


def kernel(x, packed_w, alpha, act_scale, bias, _trace=False):
    from concourse.bass_utils import run_bass_kernel_spmd

    x2d = np.asarray(x, dtype=np.float32).reshape(TOKENS, IN)
    inv_s, alpha_s = prep_scalars(alpha, act_scale)
    whost, bias_rep = prep_weights(packed_w, bias)

    nc = _build_program(inv_s, alpha_s)

    in_maps = [
        {"x": np.ascontiguousarray(x2d[c * T:(c + 1) * T]),
         "wt": whost, "bias": bias_rep}
        for c in range(N_CORES)
    ]
    res = run_bass_kernel_spmd(nc, in_maps, list(range(N_CORES)),
                               trace=_trace)

    out = np.empty((TOKENS, OUT), dtype=np.float32)
    for c in range(N_CORES):
        out[c * T:(c + 1) * T] = np.asarray(res.results[c]["out"],
                                            dtype=np.float32)
    out = out.reshape(B, S, OUT)
    if _trace:
        return out, res
    return out


# revision 12
# speedup vs baseline: 1.5835x; 1.5835x over previous
"""BitLinear (ternary-weight linear) Trainium2 kernel — fp8 DoubleRow version.

Math (matching the reference):
    s      = max(act_scale, 1e-5)
    z      = clip(round(x / s), -127, 127)           # int8-valued
    out    = (alpha * s) * (z @ sign(W).T) + bias

TRN2's fp8 DoubleRow matmul contracts 2 k-tiles (256 deep) per
instruction at 0.5 cycles per output row -> 4x bf16 MAC throughput.
z in [-127,127] is not exact in fp8e4 (4 significand bits), so k-tiles
are handled two ways:
  - EX_K exact tiles:  h   = round(z/16)  (any rounding mode works)
                       h16 = 16*h  in {-128..128 step 16} -> exact fp8e4
                       l   = z - h16  in [-15,15]         -> exact fp8e4
                       two DoubleRow passes, exact integer math in the
                       f32 PSUM accumulator.
  - SINGLE_K tiles:    s8 = fp8e4(z) single pass (~2^-4 relative rounding
                       on |z|>16). With a 16/16 split the end-to-end rel
                       err is 1.46e-2 (measured on the real inputs, incl.
                       bf16 output store) against the 2e-2 gate.
PE work: (16*2 + 16) DoubleRow instr per psum group = 328us/core.

Engine/ring plan (in-order sequencers make ring assignment = scheduling;
fill engines must not share a queue with matmul-phase consumers, or the
next rep's fill serializes behind this rep's drain):
    ACT   : quant round-scale, h-round, s8 convert   (fill only, no DMA)
    DVE   : clamp, h16 = 16h -> fp8, l = z - h16     (fill only, no DMA)
    Pool  : psum*alpha_s + bias -> bf16 drain; SWDGE ring for z stores
            and osb stores                           (matmul phase)
    SP    : x loads, xbar transposes, wt loads       (HWDGE ring)
    PE    : 3072 DoubleRow matmuls
The hl pool holds 5 quarter-slots per tag so the next rep's conversion
can start while this rep's matmuls still read older quarters.
"""

import sys

sys.path.insert(0, "/opt/trn_rl_repo")

import numpy as np
import ml_dtypes

# ---- problem constants (hardcoded per harness contract) ----
B, S, IN, OUT = 4, 4096, 4096, 4096
TOKENS = B * S              # 16384
N_CORES = 8
T = TOKENS // N_CORES       # 2048 tokens per core
KT = IN // 128              # 32 k-tiles (contraction)
EX_K = 16                   # exact k-tiles (h16 + l passes)
SINGLE_K = KT - EX_K        # single-pass k-tiles (plain fp8 quant)
N_CHUNK = 512               # output columns per PSUM tile
NT = OUT // N_CHUNK         # 8 n-chunks
Q = 512                     # token-quarter (transpose/convert granularity)
NQ = T // Q                 # 4 quarters
MT = Q // 128               # 4 m-tiles per quarter
XCH = 1024                  # free-dim chunk for quantization staging


def _build_program(inv_s: float, alpha_s: float, reps: int = 1,
                   bonly: bool = False):
    import concourse.mybir as mybir
    import concourse.tile as tile
    from concourse import bacc

    nc = bacc.Bacc("TRN2", target_bir_lowering=False, debug=False,
                   num_devices=N_CORES)

    x_d = nc.dram_tensor("x", [T, IN], mybir.dt.float32, kind="ExternalInput")
    # wt[p, n, k, c] = sign(W)[n*512 + c, k*128 + p]
    wt_d = nc.dram_tensor("wt", [128, NT, KT, N_CHUNK], mybir.dt.float8e4,
                          kind="ExternalInput")
    bias_d = nc.dram_tensor("bias", [128, OUT], mybir.dt.bfloat16,
                            kind="ExternalInput")
    out_d = nc.dram_tensor("out", [T, OUT], mybir.dt.bfloat16,
                           kind="ExternalOutput")
    z_d = nc.dram_tensor("z_scratch", [T, IN], mybir.dt.int16)

    AF = mybir.ActivationFunctionType
    ALU = mybir.AluOpType
    DR = mybir.MatmulPerfMode.DoubleRow

    with tile.TileContext(nc) as tc:
        with (
            tc.tile_pool(name="xstage", bufs=2) as xstage,
            tc.tile_pool(name="zstage", bufs=2) as zstage,
            tc.tile_pool(name="ztp", bufs=16) as ztp,
            tc.tile_pool(name="hp", bufs=4) as hp,
            tc.tile_pool(name="hl", bufs=NQ + 1) as hl_pool,
            tc.tile_pool(name="wtp", bufs=2) as wt_pool,
            tc.tile_pool(name="outsb", bufs=4) as out_pool,
            tc.tile_pool(name="biasp", bufs=1) as bias_pool,
            tc.tile_pool(name="psum", bufs=8, space="PSUM") as psum_pool,
        ):
            bias_t = bias_pool.tile([128, OUT], mybir.dt.bfloat16, tag="bias")
            nc.sync.dma_start(bias_t[:], bias_d.ap())

            def emit_quant(m):
                """Quantize one 128-token row block: x -> round/clip ->
                int16, bounce to DRAM."""
                r0 = m * 128
                for c in range(IN // XCH):
                    i0 = c * XCH
                    xt = xstage.tile([128, XCH], mybir.dt.float32, tag="xf32")
                    nc.sync.dma_start(xt[:],
                                      x_d.ap()[r0:r0 + 128, i0:i0 + XCH])
                    z0 = zstage.tile([128, XCH], mybir.dt.int16, tag="z0")
                    nc.scalar.activation(z0[:], xt[:], AF.Copy,
                                         bias=0.0, scale=float(inv_s))
                    z1 = zstage.tile([128, XCH], mybir.dt.int16, tag="z1")
                    nc.vector.tensor_scalar(z1[:], z0[:], 127.0, -127.0,
                                            ALU.min, ALU.max)
                    nc.sync.dma_start(z_d.ap()[r0:r0 + 128, i0:i0 + XCH],
                                      z1[:])

            def emit_convert(q):
                """Transpose quarter q k-major; exact tiles -> h16/l fp8,
                single-pass tiles -> s8 fp8."""
                t0 = q * Q
                h16_t = hl_pool.tile([128, EX_K, Q], mybir.dt.float8e4,
                                     tag="h16")
                l_t = hl_pool.tile([128, EX_K, Q], mybir.dt.float8e4,
                                   tag="l")
                s8_t = hl_pool.tile([128, SINGLE_K, Q], mybir.dt.float8e4,
                                    tag="s8")
                for k in range(KT):
                    zT = ztp.tile([128, Q], mybir.dt.int16, tag="zT")
                    nc.sync.dma_start_transpose(
                        zT[:], z_d.ap()[t0:t0 + Q, k * 128:(k + 1) * 128])
                    if k < EX_K:
                        h = hp.tile([128, Q], mybir.dt.int16, tag="h")
                        nc.scalar.activation(h[:], zT[:], AF.Copy,
                                             bias=0.0, scale=0.0625)
                        nc.vector.tensor_scalar(h16_t[:, k, :], h[:], 16.0,
                                                None, ALU.mult)
                        nc.vector.tensor_tensor(l_t[:, k, :], zT[:],
                                                h16_t[:, k, :], ALU.subtract)
                    else:
                        nc.scalar.activation(s8_t[:, k - EX_K, :], zT[:],
                                             AF.Copy, bias=0.0, scale=1.0)
                return h16_t, l_t, s8_t

            def load_wt(n):
                wt = wt_pool.tile([128, KT, N_CHUNK], mybir.dt.float8e4,
                                  tag="wt")
                nc.sync.dma_start(wt[:], wt_d.ap()[:, n, :, :])
                return wt

            hls = None
            for _rep in range(reps):
                wts = {0: load_wt(0), 1: load_wt(1)}
                if hls is None or not bonly:
                    hls = []
                    for q in range(NQ):
                        for mm in range(MT):
                            emit_quant(q * MT + mm)
                        hls.append(emit_convert(q))

                for n in range(NT):
                    wt = wts.pop(n)
                    for q in range(NQ):
                        h16_t, l_t, s8_t = hls[q]
                        for mm in range(MT):
                            ms = slice(mm * 128, (mm + 1) * 128)
                            psum = psum_pool.tile([128, N_CHUNK],
                                                  mybir.dt.float32)
                            for kk in range(EX_K // 2):
                                ks = slice(2 * kk, 2 * kk + 2)
                                nc.tensor.matmul(
                                    psum[:], h16_t[:, ks, ms], wt[:, ks, :],
                                    start=(kk == 0), stop=False,
                                    perf_mode=DR)
                            for kk in range(EX_K // 2):
                                ks = slice(2 * kk, 2 * kk + 2)
                                nc.tensor.matmul(
                                    psum[:], l_t[:, ks, ms], wt[:, ks, :],
                                    start=False, stop=False, perf_mode=DR)
                            for kk in range(SINGLE_K // 2):
                                ks = slice(2 * kk, 2 * kk + 2)
                                kw = slice(EX_K + 2 * kk, EX_K + 2 * kk + 2)
                                nc.tensor.matmul(
                                    psum[:], s8_t[:, ks, ms], wt[:, kw, :],
                                    start=False,
                                    stop=(kk == SINGLE_K // 2 - 1),
                                    perf_mode=DR)
                            osb = out_pool.tile([128, N_CHUNK],
                                                mybir.dt.bfloat16, tag="osb")
                            nc.vector.scalar_tensor_tensor(
                                osb[:], psum[:], float(alpha_s),
                                bias_t[:, n * N_CHUNK:(n + 1) * N_CHUNK],
                                ALU.mult, ALU.add)
                            t0 = q * Q + mm * 128
                            nc.gpsimd.dma_start(
                                out_d.ap()[t0:t0 + 128,
                                           n * N_CHUNK:(n + 1) * N_CHUNK],
                                osb[:])
                    if n + 2 < NT:
                        wts[n + 2] = load_wt(n + 2)

    nc.compile()
    return nc


def prep_scalars(alpha, act_scale):
    s = max(float(np.asarray(act_scale)), 1e-5)
    inv_s = 1.0 / np.float32(s)
    alpha_s = float(np.float32(np.asarray(alpha, dtype=np.float32)) *
                    np.float32(s))
    return float(inv_s), alpha_s


def prep_weights(packed_w, bias):
    """Host-side weight/bias packing (replicated across cores)."""
    w_sign = np.asarray(packed_w, dtype=np.float32) - 1.0     # [OUT, IN]
    # wt[p, n, k, c] = w_sign[n*512 + c, k*128 + p]
    wt = w_sign.reshape(NT, N_CHUNK, KT, 128).transpose(3, 0, 2, 1)
    whost = np.ascontiguousarray(wt.astype(ml_dtypes.float8_e4m3))
    bias_rep = np.ascontiguousarray(
        np.broadcast_to(
            np.asarray(bias, dtype=np.float32).astype(ml_dtypes.bfloat16)
            [None, :], (128, OUT)))
    return whost, bias_rep


def kernel(x, packed_w, alpha, act_scale, bias, _trace=False):
    from concourse.bass_utils import run_bass_kernel_spmd

    x2d = np.asarray(x, dtype=np.float32).reshape(TOKENS, IN)
    inv_s, alpha_s = prep_scalars(alpha, act_scale)
    whost, bias_rep = prep_weights(packed_w, bias)

    nc = _build_program(inv_s, alpha_s)

    in_maps = [
        {"x": np.ascontiguousarray(x2d[c * T:(c + 1) * T]),
         "wt": whost, "bias": bias_rep}
        for c in range(N_CORES)
    ]
    res = run_bass_kernel_spmd(nc, in_maps, list(range(N_CORES)),
                               trace=_trace)

    out = np.empty((TOKENS, OUT), dtype=np.float32)
    for c in range(N_CORES):
        out[c * T:(c + 1) * T] = np.asarray(res.results[c]["out"],
                                            dtype=np.float32)
    out = out.reshape(B, S, OUT)
    if _trace:
        return out, res
    return out


# revision 19
# speedup vs baseline: 1.7253x; 1.0895x over previous
"""BitLinear (ternary-weight linear) Trainium2 kernel — fp8 DoubleRow version.

Math (matching the reference):
    s      = max(act_scale, 1e-5)
    z      = clip(round(x / s), -127, 127)           # int8-valued
    out    = (alpha * s) * (z @ sign(W).T) + bias

TRN2's fp8 DoubleRow matmul contracts 2 k-tiles (256 deep) per
instruction at 0.5 cycles per output row -> 4x bf16 MAC throughput.
z in [-127,127] is not exact in fp8e4 (4 significand bits), so k-tiles
are handled two ways:
  - EX_K exact tiles:  h   = round(z/16)  (any rounding mode works)
                       h16 = 16*h  in {-128..128 step 16} -> exact fp8e4
                       l   = z - h16  in [-15,15]         -> exact fp8e4
                       two DoubleRow passes, exact integer math in the
                       f32 PSUM accumulator.
  - SINGLE_K tiles:    s8 = fp8e4(z) single pass (~2^-4 relative rounding
                       on |z|>16). With a 16/16 split the end-to-end rel
                       err is 1.46e-2 (measured on the real inputs, incl.
                       bf16 output store) against the 2e-2 gate.
PE work: (16*2 + 16) DoubleRow instr per psum group = 328us/core.

Engine/ring plan (in-order sequencers make ring assignment = scheduling;
fill engines must not share a queue with matmul-phase consumers, or the
next rep's fill serializes behind this rep's drain):
    ACT   : quant round-scale, h-round, s8 convert   (fill only, no DMA)
    DVE   : clamp, h16 = 16h -> fp8, l = z - h16     (fill only, no DMA)
    Pool  : psum*alpha_s + bias -> bf16 drain; SWDGE ring for z stores
            and osb stores                           (matmul phase)
    SP    : x loads, xbar transposes, wt loads       (HWDGE ring)
    PE    : 3072 DoubleRow matmuls
The hl pool holds 5 quarter-slots per tag so the next rep's conversion
can start while this rep's matmuls still read older quarters.
"""

import sys

sys.path.insert(0, "/opt/trn_rl_repo")

import numpy as np
import ml_dtypes

# ---- problem constants (hardcoded per harness contract) ----
B, S, IN, OUT = 4, 4096, 4096, 4096
TOKENS = B * S              # 16384
N_CORES = 8
T = TOKENS // N_CORES       # 2048 tokens per core
KT = IN // 128              # 32 k-tiles (contraction)
EX_K = 16                   # exact k-tiles (h16 + l passes)
SINGLE_K = KT - EX_K        # single-pass k-tiles (plain fp8 quant)
N_CHUNK = 512               # output columns per PSUM tile
NT = OUT // N_CHUNK         # 8 n-chunks
Q = 512                     # token-quarter (transpose/convert granularity)
NQ = T // Q                 # 4 quarters
MT = Q // 128               # 4 m-tiles per quarter
XCH = 1024                  # free-dim chunk for quantization staging
PIPE_Q = 1                  # quarters of the next rep converted during B


def _build_program(inv_s: float, alpha_s: float, reps: int = 1,
                   bonly: bool = False):
    import concourse.mybir as mybir
    import concourse.tile as tile
    from concourse import bacc

    nc = bacc.Bacc("TRN2", target_bir_lowering=False, debug=False,
                   num_devices=N_CORES)

    x_d = nc.dram_tensor("x", [T, IN], mybir.dt.float32, kind="ExternalInput")
    # wt[p, n, k, c] = sign(W)[n*512 + c, k*128 + p]
    wt_d = nc.dram_tensor("wt", [128, NT, KT, N_CHUNK], mybir.dt.float8e4,
                          kind="ExternalInput")
    bias_d = nc.dram_tensor("bias", [128, OUT], mybir.dt.bfloat16,
                            kind="ExternalInput")
    out_d = nc.dram_tensor("out", [T, OUT], mybir.dt.bfloat16,
                           kind="ExternalOutput")
    z_d = nc.dram_tensor("z_scratch", [T, IN], mybir.dt.int16)

    AF = mybir.ActivationFunctionType
    ALU = mybir.AluOpType
    DR = mybir.MatmulPerfMode.DoubleRow

    with tile.TileContext(nc) as tc:
        with (
            tc.tile_pool(name="xstage", bufs=2) as xstage,
            tc.tile_pool(name="zstage", bufs=2) as zstage,
            tc.tile_pool(name="ztp", bufs=16) as ztp,
            tc.tile_pool(name="hp", bufs=4) as hp,
            tc.tile_pool(name="hl", bufs=NQ + PIPE_Q) as hl_pool,
            tc.tile_pool(name="wtp", bufs=2) as wt_pool,
            tc.tile_pool(name="outsb", bufs=4) as out_pool,
            tc.tile_pool(name="biasp", bufs=1) as bias_pool,
            tc.tile_pool(name="psum", bufs=8, space="PSUM") as psum_pool,
        ):
            bias_t = bias_pool.tile([128, OUT], mybir.dt.bfloat16, tag="bias")
            nc.sync.dma_start(bias_t[:], bias_d.ap())

            def emit_quant(m):
                """Quantize one 128-token row block: x -> round/clip ->
                int16, bounce to DRAM."""
                r0 = m * 128
                for c in range(IN // XCH):
                    i0 = c * XCH
                    xt = xstage.tile([128, XCH], mybir.dt.float32, tag="xf32")
                    nc.sync.dma_start(xt[:],
                                      x_d.ap()[r0:r0 + 128, i0:i0 + XCH])
                    z0 = zstage.tile([128, XCH], mybir.dt.int16, tag="z0")
                    nc.scalar.activation(z0[:], xt[:], AF.Copy,
                                         bias=0.0, scale=float(inv_s))
                    nc.sync.dma_start(z_d.ap()[r0:r0 + 128, i0:i0 + XCH],
                                      z0[:])

            def emit_convert(q, tiles=None, ks=None):
                """Transpose quarter q k-major; exact tiles -> h16/l fp8,
                single-pass tiles -> s8 fp8. ks limits the k range so the
                work can be interleaved into the previous rep's passes."""
                t0 = q * Q
                if tiles is None:
                    tiles = (
                        hl_pool.tile([128, EX_K, Q], mybir.dt.float8e4,
                                     tag="h16", name="h16t"),
                        hl_pool.tile([128, EX_K, Q], mybir.dt.float8e4,
                                     tag="l", name="lt"),
                        hl_pool.tile([128, SINGLE_K, Q], mybir.dt.float8e4,
                                     tag="s8", name="s8t"),
                    )
                h16_t, l_t, s8_t = tiles
                for k in (range(KT) if ks is None else ks):
                    zT = ztp.tile([128, Q], mybir.dt.int16, tag="zT")
                    nc.sync.dma_start_transpose(
                        zT[:], z_d.ap()[t0:t0 + Q, k * 128:(k + 1) * 128])
                    if k < EX_K:
                        zc = hp.tile([128, Q], mybir.dt.int16, tag="zc")
                        nc.vector.tensor_scalar(zc[:], zT[:], 127.0, -127.0,
                                                ALU.min, ALU.max)
                        h = hp.tile([128, Q], mybir.dt.int16, tag="h")
                        nc.scalar.activation(h[:], zc[:], AF.Copy,
                                             bias=0.0, scale=0.0625)
                        nc.vector.tensor_scalar(h16_t[:, k, :], h[:], 16.0,
                                                None, ALU.mult)
                        nc.vector.tensor_tensor(l_t[:, k, :], zc[:],
                                                h16_t[:, k, :], ALU.subtract)
                    else:
                        nc.vector.tensor_scalar(s8_t[:, k - EX_K, :], zT[:],
                                                127.0, -127.0,
                                                ALU.min, ALU.max)
                return (h16_t, l_t, s8_t)

            def load_wt(n):
                wt = wt_pool.tile([128, KT, N_CHUNK], mybir.dt.float8e4,
                                  tag="wt")
                nc.sync.dma_start(wt[:], wt_d.ap()[:, n, :, :])
                return wt

            hls = None
            nhls = None
            for _rep in range(reps):
                wts = {0: load_wt(0), 1: load_wt(1)}
                if hls is None:
                    hls = []
                    for q in range(NQ):
                        for mm in range(MT):
                            emit_quant(q * MT + mm)
                        hls.append(emit_convert(q))
                elif not bonly:
                    pass

                for n in range(NT):
                    wt = wts.pop(n)
                    for q in range(NQ):
                        h16_t, l_t, s8_t = hls[q]
                        for mm in range(MT):
                            ms = slice(mm * 128, (mm + 1) * 128)
                            psum = psum_pool.tile([128, N_CHUNK],
                                                  mybir.dt.float32)
                            for kk in range(EX_K // 2):
                                ks = slice(2 * kk, 2 * kk + 2)
                                nc.tensor.matmul(
                                    psum[:], h16_t[:, ks, ms], wt[:, ks, :],
                                    start=(kk == 0), stop=False,
                                    perf_mode=DR)
                            for kk in range(EX_K // 2):
                                ks = slice(2 * kk, 2 * kk + 2)
                                nc.tensor.matmul(
                                    psum[:], l_t[:, ks, ms], wt[:, ks, :],
                                    start=False, stop=False, perf_mode=DR)
                            for kk in range(SINGLE_K // 2):
                                ks = slice(2 * kk, 2 * kk + 2)
                                kw = slice(EX_K + 2 * kk, EX_K + 2 * kk + 2)
                                nc.tensor.matmul(
                                    psum[:], s8_t[:, ks, ms], wt[:, kw, :],
                                    start=False,
                                    stop=(kk == SINGLE_K // 2 - 1),
                                    perf_mode=DR)
                            osb = out_pool.tile([128, N_CHUNK],
                                                mybir.dt.bfloat16, tag="osb")
                            nc.vector.scalar_tensor_tensor(
                                osb[:], psum[:], float(alpha_s),
                                bias_t[:, n * N_CHUNK:(n + 1) * N_CHUNK],
                                ALU.mult, ALU.add)
                            t0 = q * Q + mm * 128
                            nc.gpsimd.dma_start(
                                out_d.ap()[t0:t0 + 128,
                                           n * N_CHUNK:(n + 1) * N_CHUNK],
                                osb[:])
                    if n + 2 < NT:
                        wts[n + 2] = load_wt(n + 2)
                    if not bonly and _rep + 1 < reps:
                        if n < NQ:
                            for mm in range(MT):
                                emit_quant(n * MT + mm)
                        else:
                            if nhls is None:
                                nhls = [None] * NQ
                            b = n - NQ          # 0..3
                            for qq in range(PIPE_Q):
                                nhls[qq] = emit_convert(
                                    qq, tiles=nhls[qq],
                                    ks=range(b * KT // 4,
                                             (b + 1) * KT // 4))
                if not bonly and nhls is not None:
                    for qq in range(PIPE_Q, NQ):
                        nhls[qq] = emit_convert(qq)
                    hls = nhls
                    nhls = None

    nc.compile()
    return nc


def prep_scalars(alpha, act_scale):
    s = max(float(np.asarray(act_scale)), 1e-5)
    inv_s = 1.0 / np.float32(s)
    alpha_s = float(np.float32(np.asarray(alpha, dtype=np.float32)) *
                    np.float32(s))
    return float(inv_s), alpha_s


def prep_weights(packed_w, bias):
    """Host-side weight/bias packing (replicated across cores)."""
    w_sign = np.asarray(packed_w, dtype=np.float32) - 1.0     # [OUT, IN]
    # wt[p, n, k, c] = w_sign[n*512 + c, k*128 + p]
    wt = w_sign.reshape(NT, N_CHUNK, KT, 128).transpose(3, 0, 2, 1)
    whost = np.ascontiguousarray(wt.astype(ml_dtypes.float8_e4m3))
    bias_rep = np.ascontiguousarray(
        np.broadcast_to(
            np.asarray(bias, dtype=np.float32).astype(ml_dtypes.bfloat16)
            [None, :], (128, OUT)))
    return whost, bias_rep


def kernel(x, packed_w, alpha, act_scale, bias, _trace=False):
    from concourse.bass_utils import run_bass_kernel_spmd

    x2d = np.asarray(x, dtype=np.float32).reshape(TOKENS, IN)
    inv_s, alpha_s = prep_scalars(alpha, act_scale)
    whost, bias_rep = prep_weights(packed_w, bias)

    nc = _build_program(inv_s, alpha_s)

    in_maps = [
        {"x": np.ascontiguousarray(x2d[c * T:(c + 1) * T]),
         "wt": whost, "bias": bias_rep}
        for c in range(N_CORES)
    ]
    res = run_bass_kernel_spmd(nc, in_maps, list(range(N_CORES)),
                               trace=_trace)

    out = np.empty((TOKENS, OUT), dtype=np.float32)
    for c in range(N_CORES):
        out[c * T:(c + 1) * T] = np.asarray(res.results[c]["out"],
                                            dtype=np.float32)
    out = out.reshape(B, S, OUT)
    if _trace:
        return out, res
    return out
